# revision 22
# baseline (speedup 1.0000x reference)
"""Self-contained Trainium2 Bass kernel for nn_MultiHeadAttention_75273596829862.

Sharding: 8 cores = 2 batches x 4 head-groups (3 heads each). Each core
computes QKV projections for its heads, transposed-softmax attention, and a
partial output projection out_pT[768,2048]; the host sums 4 partials per
batch and adds bo.

Layout notes:
- Host pre-transposes X -> X^T [768, 2048] and casts to bf16.
- wcat columns: [Q_h0|Q_h1 | K_h0|K_h1 | Q_h2|K_h2 | V_h0|V_h1 | V_h2|pad]
  so projection m-tiles keep per-head Q/K slices on matching partition
  halves (h1 ops run at base partition 64 via tile_position).
- scores are computed transposed (t on partitions) so softmax needs no
  P-transpose; the V~ ones-column makes the PV matmul emit the softmax
  denominator as row 64 for free.
"""

import sys

for p in ("/opt/trn_rl_repo", "/root/.axon_site/_ro/trn_rl_repo"):
    if p not in sys.path:
        sys.path.insert(0, p)

import numpy as np
import ml_dtypes

BF16 = ml_dtypes.bfloat16

# Problem constants (hardcoded per spec)
B, S, DM = 2, 2048, 768
H, HD = 12, 64
NCORES = 8
HPC = 3              # heads per core
WCOLS = 640          # packed projection columns (576 used + 64 pad)
KT = DM // 128       # 6 k-tiles of the contraction dim
ST = S // 128        # 16 t-tiles of the sequence
CH = 512             # matmul moving free-dim chunk (one PSUM bank)
NCH = S // CH        # 4 chunks of the s dimension

_compiled = None


def _build(loop_reps=0, dbg=False):
    import concourse.bacc as bacc
    import concourse.tile as tile
    import concourse.mybir as mybir
    from concourse.bass import ts, ds
    from concourse.masks import make_identity

    dt = mybir.dt
    AF = mybir.ActivationFunctionType

    nc = bacc.Bacc("TRN2", target_bir_lowering=False, debug=False)

    xqT = nc.dram_tensor("xqT", [DM, S], dt.bfloat16, kind="ExternalInput").ap()
    xkT = nc.dram_tensor("xkT", [DM, S], dt.bfloat16, kind="ExternalInput").ap()
    xvT = nc.dram_tensor("xvT", [DM, S], dt.bfloat16, kind="ExternalInput").ap()
    wcat = nc.dram_tensor("wcat", [DM, WCOLS], dt.bfloat16,
                          kind="ExternalInput").ap()
    wo = nc.dram_tensor("wo", [HPC * HD, DM], dt.bfloat16,
                        kind="ExternalInput").ap()
    bcat = nc.dram_tensor("bcat", [WCOLS], dt.float32, kind="ExternalInput").ap()
    out_pT = nc.dram_tensor("out_pT", [DM, S], dt.bfloat16,
                            kind="ExternalOutput").ap()

    with tile.TileContext(nc) as tc:
        def body():
          with tc.tile_pool(name="consts", bufs=1) as cpool:
            wc_sb = cpool.tile([128, KT * WCOLS], dt.bfloat16, tag="wc")
            nc.sync.dma_start(
                out=wc_sb.rearrange("p (k c) -> p k c", k=KT),
                in_=wcat.rearrange("(k p) c -> p k c", p=128))
            wo_sb = cpool.tile([HD, HPC * DM], dt.bfloat16, tag="wos")
            nc.sync.dma_start(
                out=wo_sb.rearrange("d (h e) -> d h e", h=HPC),
                in_=wo.rearrange("(h d) e -> d h e", d=HD))
            bqk_sb = cpool.tile([128, WCOLS // 128], dt.float32, tag="bqk")
            nc.sync.dma_start(out=bqk_sb,
                              in_=bcat.rearrange("(m p) -> p m", p=128))
            ident = cpool.tile([128, 128], dt.bfloat16, tag="ident")
            make_identity(nc, ident)
            warm = cpool.tile([1, 16], dt.float32, tag="warm")
            nc.vector.memset(warm, 0.0)
            nc.scalar.activation(warm, warm, AF.Exp, scale=1.0)

            # inputs: one DMA per k-tile, issued after weights so the first
            # projection matmuls are never blocked on late weight loads
            # qk inputs in half-tiles so the first projections finish after
            # ~3MB of stream instead of 6MB; xv full tiles, streamed last
            xq_t, xk_t, xv_t = [], [], []
            for half in range(2):
                for k in range(KT):
                    for lst, dram, nm in ((xq_t, xqT, "xq"), (xk_t, xkT, "xk")):
                        tl = cpool.tile([128, S // 2], dt.bfloat16,
                                        tag=f"{nm}{k}_{half}",
                                        name=f"{nm}{k}_{half}")
                        nc.sync.dma_start(
                            out=tl, in_=dram[ts(k, 128), ts(half, S // 2)])
                        if half == 0:
                            lst.append([tl])
                        else:
                            lst[k].append(tl)
            for k in range(KT):
                tl = cpool.tile([128, S], dt.bfloat16, tag=f"xv{k}",
                                name=f"xv{k}")
                nc.sync.dma_start(out=tl, in_=xvT[ts(k, 128), :])
                xv_t.append(tl)

            def xq_k(k, c):
                return xq_t[k][c // 2][:, ts(c % 2, CH)]

            def xk_k(k, c):
                return xk_t[k][c // 2][:, ts(c % 2, CH)]

            def xv_k(k, c):
                return xv_t[k][:, ts(c, CH)]

            def wc_k(k, c0, w):
                return wc_sb[:, k * WCOLS + c0: k * WCOLS + c0 + w]

            # projection outputs (d' on partitions)
            qk01_sb = cpool.tile([128, S], dt.bfloat16, tag="qk01")
            k01_sb = cpool.tile([128, S], dt.bfloat16, tag="k01")
            qk2_sb = cpool.tile([128, S], dt.bfloat16, tag="qk2")
            kT2_sb = cpool.tile([HD, S], dt.bfloat16, tag="kT2")
            v01_sb = cpool.tile([128, S], dt.bfloat16, tag="v01")
            v2_sb = cpool.tile([HD, S], dt.bfloat16, tag="v2")
            # V~ tiles: vv01[t] [128, 130] = [V_h0|1|V_h1|1]; vv2[t] [128, 65]
            vv01 = [cpool.tile([128, 2 * (HD + 1)], dt.bfloat16,
                               tag=f"vv01_{t}", name=f"vv01_{t}")
                    for t in range(ST)]
            vv2 = [cpool.tile([128, HD + 1], dt.bfloat16,
                              tag=f"vv2_{t}", name=f"vv2_{t}")
                   for t in range(ST)]
            onrm_sb = [cpool.tile([HD, S], dt.bfloat16, tag=f"onrm{h}",
                                  name=f"onrm{h}")
                       for h in range(HPC)]

            # ---------------- projections ----------------
            # Phase P1: V (+ Q2/K2) projections and V transposes first, so
            # attention for h0 can start as soon as Q01/K01 (phase P2) land.
            def proj_group(pool, c, col0, m, xf, row0, pi, tag=None, bufs=1):
                pt = pool.tile([128, CH], dt.float32, tag=tag or f"pp{pi}",
                               name=f"pp{pi}", bufs=bufs)
                for k in range(KT):
                    nc.tensor.matmul(
                        pt[row0:row0 + m, :],
                        wc_k(k, col0, m),
                        xf(k, c),
                        start=(k == 0),
                        stop=(k == KT - 1),
                    )
                return pt

            def bias_copy(pt, dst, c, bi):
                rows = dst.shape[0]
                nc.vector.tensor_scalar_add(
                    dst[:, ts(c, CH)], pt[0:rows, :], bqk_sb[0:rows, bi:bi + 1])

            def emit_vpath_chunk(pq1, ptr, c):
                p2 = proj_group(pq1, c, 256, 64, xq_k, 0, 2, tag="ppv",
                                bufs=2)
                for k in range(KT):
                    nc.tensor.matmul(
                        p2[64:128, :],
                        wc_k(k, 320, 64),
                        xk_k(k, c),
                        start=(k == 0),
                        stop=(k == KT - 1),
                    )
                bias_copy(p2, qk2_sb, c, 2)
                p3 = proj_group(pq1, c, 384, 128, xv_k, 0, 3, tag="ppv",
                                bufs=2)
                bias_copy(p3, v01_sb, c, 3)
                p4 = proj_group(pq1, c, 512, 64, xv_k, 0, 4, tag="ppv",
                                bufs=2)
                bias_copy(p4, v2_sb, c, 4)
                for t in range(4 * c, 4 * c + 4):
                    tr1 = ptr.tile([128, 128], dt.bfloat16, tag="ppv",
                                   name="tr1", bufs=2)
                    nc.tensor.transpose(tr1, v01_sb[:, ts(t, 128)], ident)
                    nc.vector.tensor_copy(
                        vv01[t].rearrange("p (h x) -> p h x",
                                          h=2)[:, :, 0:HD],
                        tr1.rearrange("p (h x) -> p h x", h=2),
                    )
                    nc.gpsimd.memset(
                        vv01[t].rearrange("p (h x) -> p h x",
                                          h=2)[:, :, HD:HD + 1],
                        1.0)
                    tr2 = ptr.tile([128, HD], dt.bfloat16, tag="ppv",
                                   name="tr2", bufs=2)
                    nc.tensor.transpose(tr2, v2_sb[:, ts(t, 128)],
                                        ident[0:HD, 0:HD])
                    nc.vector.tensor_copy(vv2[t][:, 0:HD], tr2)
                    nc.gpsimd.memset(vv2[t][:, HD:HD + 1], 1.0)

            qk_heads = [
                (qk01_sb[0:HD, :], k01_sb[0:HD, :]),
                (qk01_sb[HD:128, :], k01_sb[HD:128, :]),
                (qk2_sb[0:HD, :], kT2_sb),
            ]
            vv_heads = [
                lambda t: vv01[t][:, 0:HD + 1],
                lambda t: vv01[t][:, HD + 1:2 * (HD + 1)],
                lambda t: vv2[t],
            ]

            # Phase A: QK01 projections interleaved with head-0 half-0
            # scores/exp (ACT starts while projections still stream)
            def sc_exp(h, t, half, psp, epool, ps_bufs=2, etag="exp",
                       ebufs=4):
                qT_h, kT_h = qk_heads[h]
                ps = psp.tile([128, 2 * CH], dt.float32, tag="ps", name="ps",
                              bufs=ps_bufs)
                for j in range(2):
                    c = 2 * half + j
                    nc.tensor.matmul(
                        ps[:, ts(j, CH)],
                        kT_h[:, ts(t, 128)],
                        qT_h[:, ts(c, CH)],
                    )
                et = epool.tile([128, 2 * CH], dt.bfloat16, tag=etag,
                                name="et", bufs=ebufs)
                nc.scalar.activation(et, ps, AF.Exp, scale=0.125)
                return et

            def pv(h, t, half, et, po, start=None, stop=None):
                for j in range(2):
                    c = 2 * half + j
                    nc.tensor.matmul(
                        po[c],
                        vv_heads[h](t),
                        et[:, ts(j, CH)],
                        start=(t == 0) if start is None else start,
                        stop=(t == ST - 1) if stop is None else stop,
                    )

            def norm_head(h, po, spool):
                for c in range(NCH):
                    dtile = spool.tile([1, CH], dt.float32, tag="den",
                                       name="dtile")
                    nc.vector.tensor_copy(dtile, po[c][HD:HD + 1, :])
                    rtile = spool.tile([1, CH], dt.float32, tag="rec",
                                       name="rtile")
                    nc.vector.reciprocal_approx_fast(out=rtile, in_=dtile)
                    bcst = spool.tile([HD, CH], dt.float32, tag="bcast",
                                      name="bcst")
                    nc.gpsimd.partition_broadcast(bcst, rtile)
                    dst = onrm_sb[h][:, ts(c, CH)]
                    nc.vector.tensor_mul(dst, po[c][0:HD, :], bcst)

            ets0 = {}
            with (
                tc.tile_pool(name="pproj2", bufs=1, space="PSUM") as pq2,
                tc.tile_pool(name="psA", bufs=1, space="PSUM") as psA,
                tc.tile_pool(name="expA", bufs=16) as epoolA,
            ):
                ready = {1: range(0, 4), 2: range(4, 8), 3: range(8, 16)}
                for c in range(NCH):
                    p0 = proj_group(pq2, c, 0, 128, xq_k, 0, 0)
                    bias_copy(p0, qk01_sb, c, 0)
                    p1 = proj_group(pq2, c, 128, 128, xk_k, 0, 1)
                    bias_copy(p1, k01_sb, c, 1)
                    for t in ready.get(c, ()):
                        ets0[t] = sc_exp(0, t, 0, psA, epoolA, ps_bufs=2,
                                         etag="expA", ebufs=16)

            def norm_chunk(h, c, po_c, spool):
                dtile = spool.tile([1, CH], dt.float32, tag="den",
                                   name="dtile")
                nc.vector.tensor_copy(dtile, po_c[HD:HD + 1, :])
                rtile = spool.tile([1, CH], dt.float32, tag="rec",
                                   name="rtile")
                nc.vector.reciprocal_approx_fast(out=rtile, in_=dtile)
                bcst = spool.tile([HD, CH], dt.float32, tag="bcast",
                                  name="bcst")
                nc.gpsimd.partition_broadcast(bcst, rtile)
                nc.vector.tensor_mul(onrm_sb[h][:, ts(c, CH)],
                                     po_c[0:HD, :], bcst)

            def pv_half(h, t, half, et, poA, poB):
                for j, po_c in ((0, poA), (1, poB)):
                    nc.tensor.matmul(
                        po_c,
                        vv_heads[h](t),
                        et[:, ts(j, CH)],
                        start=(t == 0),
                        stop=(t == ST - 1),
                    )

            def outproj_chunk(c, pool, opool_t):
                for e in range(KT):
                    pout = pool.tile([128, CH], dt.float32, tag="pAB",
                                     name="pout", bufs=2)
                    for h in range(HPC):
                        nc.tensor.matmul(
                            pout,
                            wo_sb[:, h * DM + e * 128: h * DM + (e + 1) * 128],
                            onrm_sb[h][:, ts(c, CH)],
                            start=(h == 0),
                            stop=(h == HPC - 1),
                        )
                        
                    if c % 2 == 0:
                        nc.vector.tensor_copy(opool_t[e][:, ts(c, CH)], pout)
                    else:
                        nc.scalar.copy(opool_t[e][:, ts(c, CH)], pout)
                    if c == NCH - 1:
                        nc.sync.dma_start(out=out_pT[ts(e, 128), :],
                                          in_=opool_t[e])

            # output staging tiles (written chunk-wise, DMA'd when complete)
            ot_tiles = [cpool.tile([128, S], dt.bfloat16, tag=f"ot{e}",
                                   name=f"ot{e}") for e in range(KT)]

            # Phase B: V-path + head-0 half-0 PV + its norms
            with (
                tc.tile_pool(name="pvb", bufs=1, space="PSUM") as pvb,
                tc.tile_pool(name="smallsB", bufs=4) as spoolB,
            ):
                po00 = pvb.tile([HD + 1, CH], dt.float32, tag="po0",
                                name="po00")
                po01 = pvb.tile([HD + 1, CH], dt.float32, tag="po1",
                                name="po01")
                with tc.tile_pool(name="ppv", bufs=1, space="PSUM") as ppvp:
                    for c in range(NCH):
                        emit_vpath_chunk(ppvp, ppvp, c)
                        for t in range(4 * c, 4 * c + 4):
                            pv_half(0, t, 0, ets0.pop(t), po00, po01)
                    nc.sync.dma_start(out=kT2_sb, in_=qk2_sb[64:128, :])
                norm_chunk(0, 0, po00, spoolB)
                norm_chunk(0, 1, po01, spoolB)

            # Phases C/D: software-pipelined half-major attention
            with (
                tc.tile_pool(name="pCD", bufs=1, space="PSUM") as pcd,
                tc.tile_pool(name="expCD", bufs=5) as epool,
                tc.tile_pool(name="smalls", bufs=4) as spool,
            ):
                def half_loop(h, half):
                    poA = pcd.tile([HD + 1, CH], dt.float32, tag="pAB",
                                   name="poA", bufs=2)
                    poB = pcd.tile([HD + 1, CH], dt.float32, tag="pAB",
                                   name="poB", bufs=2)
                    ets = {}
                    for t in range(ST):
                        ets[t] = sc_exp(h, t, half, pcd, epool, ps_bufs=3,
                                        etag="exp", ebufs=5)
                        if t >= 2:
                            pv_half(h, t - 2, half, ets.pop(t - 2), poA, poB)
                    pv_half(h, ST - 2, half, ets.pop(ST - 2), poA, poB)
                    pv_half(h, ST - 1, half, ets.pop(ST - 1), poA, poB)
                    norm_chunk(h, 2 * half, poA, spool)
                    norm_chunk(h, 2 * half + 1, poB, spool)

                half_loop(0, 1)
                half_loop(1, 0)
                half_loop(1, 1)
                half_loop(2, 0)
                # overlap first half of output projection with h2's second half
                outproj_chunk(0, pcd, ot_tiles)
                half_loop(2, 1)
                outproj_chunk(1, pcd, ot_tiles)
                outproj_chunk(2, pcd, ot_tiles)
                outproj_chunk(3, pcd, ot_tiles)

        if loop_reps > 1:
            with tc.For_i(0, loop_reps, 1):
                body()
        else:
            body()

    nc.compile()
    return nc


def _shard_inputs(query, key, value, wq, bq, wk, bk, wv, bv, wo, bo):
    """Build the 8 per-core input maps."""
    f32 = np.float32
    in_maps = []
    for core in range(NCORES):
        b = core // 4
        h0 = (core % 4) * HPC
        cs = slice(h0 * HD, (h0 + HPC) * HD)
        wq_s = np.asarray(wq[:, cs], f32)
        wk_s = np.asarray(wk[:, cs], f32)
        wv_s = np.asarray(wv[:, cs], f32)
        pad = np.zeros((DM, HD), f32)
        wcat = np.concatenate(
            [wq_s[:, 0:128], wk_s[:, 0:128], wq_s[:, 128:192],
             wk_s[:, 128:192], wv_s[:, 0:128], wv_s[:, 128:192], pad], axis=1)
        bq_s, bk_s, bv_s = (np.asarray(x[cs], f32) for x in (bq, bk, bv))
        bcat = np.concatenate([bq_s[0:128], bk_s[0:128], bq_s[128:192],
                               bk_s[128:192], bv_s[0:128], bv_s[128:192],
                               np.zeros(64, f32)])
        in_maps.append({
            "xqT": np.ascontiguousarray(np.asarray(query, f32)[b].T).astype(BF16),
            "xkT": np.ascontiguousarray(np.asarray(key, f32)[b].T).astype(BF16),
            "xvT": np.ascontiguousarray(np.asarray(value, f32)[b].T).astype(BF16),
            "wcat": np.ascontiguousarray(wcat).astype(BF16),
            "wo": np.ascontiguousarray(np.asarray(wo, f32)[cs, :]).astype(BF16),
            "bcat": np.ascontiguousarray(bcat),
        })
    return in_maps


def kernel(query, key, value, wq, bq, wk, bk, wv, bv, wo, bo):
    global _compiled
    from concourse.bass_utils import run_bass_kernel_spmd

    if _compiled is None:
        _compiled = _build()
    nc = _compiled

    in_maps = _shard_inputs(query, key, value, wq, bq, wk, bk, wv, bv, wo, bo)
    res = run_bass_kernel_spmd(nc, in_maps, list(range(NCORES)))

    out = np.zeros((B, S, DM), dtype=np.float32)
    for core in range(NCORES):
        b = core // 4
        out[b] += res.results[core]["out_pT"].astype(np.float32).T
    corr = (np.asarray(bv, np.float64) @ np.asarray(wo, np.float64)
            + np.asarray(bo, np.float64)).astype(np.float32)
    out += corr[None, None, :]
    return out


# revision 24
# speedup vs baseline: 2.4975x; 2.4975x over previous
"""Self-contained Trainium2 Bass kernel for nn_MultiHeadAttention_75273596829862.

Sharding: 8 cores = 2 batches x 4 head-groups (3 heads each). Each core
computes QKV projections for its heads, transposed-softmax attention, and a
partial output projection out_pT[768,2048]; the host sums 4 partials per
batch and adds bo.

Layout notes:
- Host pre-transposes X -> X^T [768, 2048] and casts to bf16.
- wcat columns: [Q_h0|Q_h1 | K_h0|K_h1 | Q_h2|K_h2 | V_h0|V_h1 | V_h2|pad]
  so projection m-tiles keep per-head Q/K slices on matching partition
  halves (h1 ops run at base partition 64 via tile_position).
- scores are computed transposed (t on partitions) so softmax needs no
  P-transpose; the V~ ones-column makes the PV matmul emit the softmax
  denominator as row 64 for free.
"""

import sys

for p in ("/opt/trn_rl_repo", "/root/.axon_site/_ro/trn_rl_repo"):
    if p not in sys.path:
        sys.path.insert(0, p)

import numpy as np
import ml_dtypes

BF16 = ml_dtypes.bfloat16

# Problem constants (hardcoded per spec)
B, S, DM = 2, 2048, 768
H, HD = 12, 64
NCORES = 8
HPC = 3              # heads per core
WCOLS = 640          # packed projection columns (576 used + 64 pad)
KT = DM // 128       # 6 k-tiles of the contraction dim
ST = S // 128        # 16 t-tiles of the sequence
CH = 512             # matmul moving free-dim chunk (one PSUM bank)
NCH = S // CH        # 4 chunks of the s dimension

_compiled = None


def _build(loop_reps=0, dbg=False):
    import concourse.bacc as bacc
    import concourse.tile as tile
    import concourse.mybir as mybir
    from concourse.bass import ts, ds
    from concourse.masks import make_identity

    dt = mybir.dt
    AF = mybir.ActivationFunctionType

    nc = bacc.Bacc("TRN2", target_bir_lowering=False, debug=False)

    xqT = nc.dram_tensor("xqT", [DM, S], dt.bfloat16, kind="ExternalInput").ap()
    xkT = nc.dram_tensor("xkT", [DM, S], dt.bfloat16, kind="ExternalInput").ap()
    xvT = nc.dram_tensor("xvT", [DM, S], dt.bfloat16, kind="ExternalInput").ap()
    wcat = nc.dram_tensor("wcat", [DM, WCOLS], dt.bfloat16,
                          kind="ExternalInput").ap()
    wo = nc.dram_tensor("wo", [HPC * HD, DM], dt.bfloat16,
                        kind="ExternalInput").ap()
    bcat = nc.dram_tensor("bcat", [WCOLS], dt.float32, kind="ExternalInput").ap()
    out_pT = nc.dram_tensor("out_pT", [DM, S], dt.bfloat16,
                            kind="ExternalOutput").ap()

    with tile.TileContext(nc) as tc:
        def body():
          with tc.tile_pool(name="consts", bufs=1) as cpool:
            wc_sb = cpool.tile([128, KT * WCOLS], dt.bfloat16, tag="wc")
            nc.sync.dma_start(
                out=wc_sb.rearrange("p (k c) -> p k c", k=KT),
                in_=wcat.rearrange("(k p) c -> p k c", p=128))
            wo_sb = cpool.tile([HD, HPC * DM], dt.bfloat16, tag="wos")
            nc.sync.dma_start(
                out=wo_sb.rearrange("d (h e) -> d h e", h=HPC),
                in_=wo.rearrange("(h d) e -> d h e", d=HD))
            bqk_sb = cpool.tile([128, WCOLS // 128], dt.float32, tag="bqk")
            nc.sync.dma_start(out=bqk_sb,
                              in_=bcat.rearrange("(m p) -> p m", p=128))
            ident = cpool.tile([128, 128], dt.bfloat16, tag="ident")
            make_identity(nc, ident)
            warm = cpool.tile([1, 16], dt.float32, tag="warm")
            nc.vector.memset(warm, 0.0)
            nc.scalar.activation(warm, warm, AF.Exp, scale=1.0)

            # inputs: one DMA per k-tile, issued after weights so the first
            # projection matmuls are never blocked on late weight loads
            # qk inputs in half-tiles so the first projections finish after
            # ~3MB of stream instead of 6MB; xv full tiles, streamed last
            xq_t, xk_t, xv_t = [], [], []
            for half in range(2):
                for k in range(KT):
                    for lst, dram, nm in ((xq_t, xqT, "xq"), (xk_t, xkT, "xk")):
                        tl = cpool.tile([128, S // 2], dt.bfloat16,
                                        tag=f"{nm}{k}_{half}",
                                        name=f"{nm}{k}_{half}")
                        nc.sync.dma_start(
                            out=tl, in_=dram[ts(k, 128), ts(half, S // 2)])
                        if half == 0:
                            lst.append([tl])
                        else:
                            lst[k].append(tl)
            for k in range(KT):
                tl = cpool.tile([128, S], dt.bfloat16, tag=f"xv{k}",
                                name=f"xv{k}")
                nc.sync.dma_start(out=tl, in_=xvT[ts(k, 128), :])
                xv_t.append(tl)

            def xq_k(k, c):
                return xq_t[k][c // 2][:, ts(c % 2, CH)]

            def xk_k(k, c):
                return xk_t[k][c // 2][:, ts(c % 2, CH)]

            def xv_k(k, c):
                return xv_t[k][:, ts(c, CH)]

            def wc_k(k, c0, w):
                return wc_sb[:, k * WCOLS + c0: k * WCOLS + c0 + w]

            # projection outputs (d' on partitions)
            qk01_sb = cpool.tile([128, S], dt.bfloat16, tag="qk01")
            k01_sb = cpool.tile([128, S], dt.bfloat16, tag="k01")
            qk2_sb = cpool.tile([128, S], dt.bfloat16, tag="qk2")
            kT2_sb = cpool.tile([HD, S], dt.bfloat16, tag="kT2")
            v01_sb = cpool.tile([128, S], dt.bfloat16, tag="v01")
            v2_sb = cpool.tile([HD, S], dt.bfloat16, tag="v2")
            # V~ tiles: vv01[t] [128, 130] = [V_h0|1|V_h1|1]; vv2[t] [128, 65]
            vv01 = [cpool.tile([128, 2 * (HD + 1)], dt.bfloat16,
                               tag=f"vv01_{t}", name=f"vv01_{t}")
                    for t in range(ST)]
            vv2 = [cpool.tile([128, HD + 1], dt.bfloat16,
                              tag=f"vv2_{t}", name=f"vv2_{t}")
                   for t in range(ST)]
            onrm_sb = [cpool.tile([HD, S], dt.bfloat16, tag=f"onrm{h}",
                                  name=f"onrm{h}")
                       for h in range(HPC)]

            # ---------------- projections ----------------
            # Phase P1: V (+ Q2/K2) projections and V transposes first, so
            # attention for h0 can start as soon as Q01/K01 (phase P2) land.
            def proj_group(pool, c, col0, m, xf, row0, pi, tag=None, bufs=1):
                pt = pool.tile([128, CH], dt.float32, tag=tag or f"pp{pi}",
                               name=f"pp{pi}", bufs=bufs)
                for k in range(KT):
                    nc.tensor.matmul(
                        pt[row0:row0 + m, :],
                        wc_k(k, col0, m),
                        xf(k, c),
                        start=(k == 0),
                        stop=(k == KT - 1),
                    )
                return pt

            def bias_copy(pt, dst, c, bi):
                rows = dst.shape[0]
                nc.vector.tensor_scalar_add(
                    dst[:, ts(c, CH)], pt[0:rows, :], bqk_sb[0:rows, bi:bi + 1])

            def emit_vpath_chunk(pq1, ptr, c):
                p2 = proj_group(pq1, c, 256, 64, xq_k, 0, 2, tag="ppv",
                                bufs=2)
                for k in range(KT):
                    nc.tensor.matmul(
                        p2[64:128, :],
                        wc_k(k, 320, 64),
                        xk_k(k, c),
                        start=(k == 0),
                        stop=(k == KT - 1),
                    )
                bias_copy(p2, qk2_sb, c, 2)
                p3 = proj_group(pq1, c, 384, 128, xv_k, 0, 3, tag="ppv",
                                bufs=2)
                bias_copy(p3, v01_sb, c, 3)
                p4 = proj_group(pq1, c, 512, 64, xv_k, 0, 4, tag="ppv",
                                bufs=2)
                bias_copy(p4, v2_sb, c, 4)
                for t in range(4 * c, 4 * c + 4):
                    tr1 = ptr.tile([128, 128], dt.bfloat16, tag="ppv",
                                   name="tr1", bufs=2)
                    nc.tensor.transpose(tr1, v01_sb[:, ts(t, 128)], ident)
                    nc.vector.tensor_copy(
                        vv01[t].rearrange("p (h x) -> p h x",
                                          h=2)[:, :, 0:HD],
                        tr1.rearrange("p (h x) -> p h x", h=2),
                    )
                    nc.gpsimd.memset(
                        vv01[t].rearrange("p (h x) -> p h x",
                                          h=2)[:, :, HD:HD + 1],
                        1.0)
                    tr2 = ptr.tile([128, HD], dt.bfloat16, tag="ppv",
                                   name="tr2", bufs=2)
                    nc.tensor.transpose(tr2, v2_sb[:, ts(t, 128)],
                                        ident[0:HD, 0:HD])
                    nc.vector.tensor_copy(vv2[t][:, 0:HD], tr2)
                    nc.gpsimd.memset(vv2[t][:, HD:HD + 1], 1.0)

            qk_heads = [
                (qk01_sb[0:HD, :], k01_sb[0:HD, :]),
                (qk01_sb[HD:128, :], k01_sb[HD:128, :]),
                (qk2_sb[0:HD, :], kT2_sb),
            ]
            vv_heads = [
                lambda t: vv01[t][:, 0:HD + 1],
                lambda t: vv01[t][:, HD + 1:2 * (HD + 1)],
                lambda t: vv2[t],
            ]

            # Phase A: QK01 projections interleaved with head-0 half-0
            # scores/exp (ACT starts while projections still stream)
            def sc_exp(h, t, half, psp, epool, ps_bufs=2, etag="exp",
                       ebufs=4):
                qT_h, kT_h = qk_heads[h]
                ps = psp.tile([128, 2 * CH], dt.float32, tag="ps", name="ps",
                              bufs=ps_bufs)
                for j in range(2):
                    c = 2 * half + j
                    nc.tensor.matmul(
                        ps[:, ts(j, CH)],
                        kT_h[:, ts(t, 128)],
                        qT_h[:, ts(c, CH)],
                    )
                et = epool.tile([128, 2 * CH], dt.bfloat16, tag=etag,
                                name="et", bufs=ebufs)
                nc.scalar.activation(et, ps, AF.Exp, scale=0.125)
                return et

            def pv(h, t, half, et, po, start=None, stop=None):
                for j in range(2):
                    c = 2 * half + j
                    nc.tensor.matmul(
                        po[c],
                        vv_heads[h](t),
                        et[:, ts(j, CH)],
                        start=(t == 0) if start is None else start,
                        stop=(t == ST - 1) if stop is None else stop,
                    )

            def norm_head(h, po, spool):
                for c in range(NCH):
                    dtile = spool.tile([1, CH], dt.float32, tag="den",
                                       name="dtile")
                    nc.vector.tensor_copy(dtile, po[c][HD:HD + 1, :])
                    rtile = spool.tile([1, CH], dt.float32, tag="rec",
                                       name="rtile")
                    nc.vector.reciprocal_approx_fast(out=rtile, in_=dtile)
                    bcst = spool.tile([HD, CH], dt.float32, tag="bcast",
                                      name="bcst")
                    nc.gpsimd.partition_broadcast(bcst, rtile)
                    dst = onrm_sb[h][:, ts(c, CH)]
                    nc.vector.tensor_mul(dst, po[c][0:HD, :], bcst)

            ets0 = {}
            with (
                tc.tile_pool(name="pproj2", bufs=1, space="PSUM") as pq2,
                tc.tile_pool(name="psA", bufs=1, space="PSUM") as psA,
                tc.tile_pool(name="expA", bufs=16) as epoolA,
            ):
                ready = {1: range(0, 4), 2: range(4, 8), 3: range(8, 16)}
                for c in range(NCH):
                    p0 = proj_group(pq2, c, 0, 128, xq_k, 0, 0)
                    bias_copy(p0, qk01_sb, c, 0)
                    p1 = proj_group(pq2, c, 128, 128, xk_k, 0, 1)
                    bias_copy(p1, k01_sb, c, 1)
                    for t in ready.get(c, ()):
                        ets0[t] = sc_exp(0, t, 0, psA, epoolA, ps_bufs=3,
                                         etag="expA", ebufs=16)

            def norm_chunk(h, c, po_c, spool):
                dtile = spool.tile([1, CH], dt.float32, tag="den",
                                   name="dtile")
                nc.vector.tensor_copy(dtile, po_c[HD:HD + 1, :])
                rtile = spool.tile([1, CH], dt.float32, tag="rec",
                                   name="rtile")
                nc.vector.reciprocal_approx_fast(out=rtile, in_=dtile)
                bcst = spool.tile([HD, CH], dt.float32, tag="bcast",
                                  name="bcst")
                nc.gpsimd.partition_broadcast(bcst, rtile)
                nc.vector.tensor_mul(onrm_sb[h][:, ts(c, CH)],
                                     po_c[0:HD, :], bcst)

            def pv_half(h, t, half, et, poA, poB):
                for j, po_c in ((0, poA), (1, poB)):
                    nc.tensor.matmul(
                        po_c,
                        vv_heads[h](t),
                        et[:, ts(j, CH)],
                        start=(t == 0),
                        stop=(t == ST - 1),
                    )

            def outproj_chunk(c, pool, opool_t):
                for e in range(KT):
                    pout = pool.tile([128, CH], dt.float32, tag="pAB",
                                     name="pout", bufs=2)
                    for h in range(HPC):
                        nc.tensor.matmul(
                            pout,
                            wo_sb[:, h * DM + e * 128: h * DM + (e + 1) * 128],
                            onrm_sb[h][:, ts(c, CH)],
                            start=(h == 0),
                            stop=(h == HPC - 1),
                        )
                        
                    if c % 2 == 0:
                        nc.vector.tensor_copy(opool_t[e][:, ts(c, CH)], pout)
                    else:
                        nc.scalar.copy(opool_t[e][:, ts(c, CH)], pout)
                    if c == NCH - 1:
                        nc.sync.dma_start(out=out_pT[ts(e, 128), :],
                                          in_=opool_t[e])

            # output staging tiles (written chunk-wise, DMA'd when complete)
            ot_tiles = [cpool.tile([128, S], dt.bfloat16, tag=f"ot{e}",
                                   name=f"ot{e}") for e in range(KT)]

            # Phase B: V-path + head-0 half-0 PV + its norms
            with (
                tc.tile_pool(name="pvb", bufs=1, space="PSUM") as pvb,
                tc.tile_pool(name="smallsB", bufs=4) as spoolB,
            ):
                po00 = pvb.tile([HD + 1, CH], dt.float32, tag="po0",
                                name="po00")
                po01 = pvb.tile([HD + 1, CH], dt.float32, tag="po1",
                                name="po01")
                with tc.tile_pool(name="ppv", bufs=1, space="PSUM") as ppvp:
                    for c in range(NCH):
                        emit_vpath_chunk(ppvp, ppvp, c)
                        for t in range(4 * c, 4 * c + 4):
                            pv_half(0, t, 0, ets0.pop(t), po00, po01)
                    nc.sync.dma_start(out=kT2_sb, in_=qk2_sb[64:128, :])
                norm_chunk(0, 0, po00, spoolB)
                norm_chunk(0, 1, po01, spoolB)

            # Phases C/D: software-pipelined half-major attention
            with (
                tc.tile_pool(name="pCD", bufs=1, space="PSUM") as pcd,
                tc.tile_pool(name="expCD", bufs=8) as epool,
                tc.tile_pool(name="smalls", bufs=4) as spool,
            ):
                def half_loop(h, half):
                    poA = pcd.tile([HD + 1, CH], dt.float32, tag="pAB",
                                   name="poA", bufs=2)
                    poB = pcd.tile([HD + 1, CH], dt.float32, tag="pAB",
                                   name="poB", bufs=2)
                    ets = {}
                    LAG = 3
                    for t in range(ST):
                        ets[t] = sc_exp(h, t, half, pcd, epool, ps_bufs=3,
                                        etag="exp", ebufs=8)
                        if t >= LAG:
                            pv_half(h, t - LAG, half, ets.pop(t - LAG),
                                    poA, poB)
                    for t in range(ST - LAG, ST):
                        pv_half(h, t, half, ets.pop(t), poA, poB)
                    norm_chunk(h, 2 * half, poA, spool)
                    norm_chunk(h, 2 * half + 1, poB, spool)

                half_loop(0, 1)
                half_loop(1, 0)
                half_loop(1, 1)
                half_loop(2, 0)
                # overlap first half of output projection with h2's second half
                outproj_chunk(0, pcd, ot_tiles)
                half_loop(2, 1)
                outproj_chunk(1, pcd, ot_tiles)
                outproj_chunk(2, pcd, ot_tiles)
                outproj_chunk(3, pcd, ot_tiles)

        if loop_reps > 1:
            with tc.For_i(0, loop_reps, 1):
                body()
        else:
            body()

    nc.compile()
    return nc


def _shard_inputs(query, key, value, wq, bq, wk, bk, wv, bv, wo, bo):
    """Build the 8 per-core input maps."""
    f32 = np.float32
    in_maps = []
    for core in range(NCORES):
        b = core // 4
        h0 = (core % 4) * HPC
        cs = slice(h0 * HD, (h0 + HPC) * HD)
        wq_s = np.asarray(wq[:, cs], f32)
        wk_s = np.asarray(wk[:, cs], f32)
        wv_s = np.asarray(wv[:, cs], f32)
        pad = np.zeros((DM, HD), f32)
        wcat = np.concatenate(
            [wq_s[:, 0:128], wk_s[:, 0:128], wq_s[:, 128:192],
             wk_s[:, 128:192], wv_s[:, 0:128], wv_s[:, 128:192], pad], axis=1)
        bq_s, bk_s, bv_s = (np.asarray(x[cs], f32) for x in (bq, bk, bv))
        bcat = np.concatenate([bq_s[0:128], bk_s[0:128], bq_s[128:192],
                               bk_s[128:192], bv_s[0:128], bv_s[128:192],
                               np.zeros(64, f32)])
        in_maps.append({
            "xqT": np.ascontiguousarray(np.asarray(query, f32)[b].T).astype(BF16),
            "xkT": np.ascontiguousarray(np.asarray(key, f32)[b].T).astype(BF16),
            "xvT": np.ascontiguousarray(np.asarray(value, f32)[b].T).astype(BF16),
            "wcat": np.ascontiguousarray(wcat).astype(BF16),
            "wo": np.ascontiguousarray(np.asarray(wo, f32)[cs, :]).astype(BF16),
            "bcat": np.ascontiguousarray(bcat),
        })
    return in_maps


def kernel(query, key, value, wq, bq, wk, bk, wv, bv, wo, bo):
    global _compiled
    from concourse.bass_utils import run_bass_kernel_spmd

    if _compiled is None:
        _compiled = _build()
    nc = _compiled

    in_maps = _shard_inputs(query, key, value, wq, bq, wk, bk, wv, bv, wo, bo)
    res = run_bass_kernel_spmd(nc, in_maps, list(range(NCORES)))

    out = np.zeros((B, S, DM), dtype=np.float32)
    for core in range(NCORES):
        b = core // 4
        out[b] += res.results[core]["out_pT"].astype(np.float32).T
    corr = (np.asarray(bv, np.float64) @ np.asarray(wo, np.float64)
            + np.asarray(bo, np.float64)).astype(np.float32)
    out += corr[None, None, :]
    return out


# revision 28
# speedup vs baseline: 3.1263x; 1.2518x over previous
"""Self-contained Trainium2 Bass kernel for nn_MultiHeadAttention_75273596829862.

Sharding: 8 cores = 2 batches x 4 head-groups (3 heads each). Each core
computes QKV projections for its heads, transposed-softmax attention, and a
partial output projection out_pT[768,2048]; the host sums 4 partials per
batch and adds bo.

Layout notes:
- Host pre-transposes X -> X^T [768, 2048] and casts to bf16.
- wcat columns: [Q_h0|Q_h1 | K_h0|K_h1 | Q_h2|K_h2 | V_h0|V_h1 | V_h2|pad]
  so projection m-tiles keep per-head Q/K slices on matching partition
  halves (h1 ops run at base partition 64 via tile_position).
- scores are computed transposed (t on partitions) so softmax needs no
  P-transpose; the V~ ones-column makes the PV matmul emit the softmax
  denominator as row 64 for free.
"""

import sys

for p in ("/opt/trn_rl_repo", "/root/.axon_site/_ro/trn_rl_repo"):
    if p not in sys.path:
        sys.path.insert(0, p)

import numpy as np
import ml_dtypes

BF16 = ml_dtypes.bfloat16

# Problem constants (hardcoded per spec)
B, S, DM = 2, 2048, 768
H, HD = 12, 64
NCORES = 8
HPC = 3              # heads per core
WCOLS = 640          # packed projection columns (576 used + 64 pad)
KT = DM // 128       # 6 k-tiles of the contraction dim
ST = S // 128        # 16 t-tiles of the sequence
CH = 512             # matmul moving free-dim chunk (one PSUM bank)
NCH = S // CH        # 4 chunks of the s dimension

_compiled = None


def _build(loop_reps=0, dbg=False):
    import concourse.bacc as bacc
    import concourse.tile as tile
    import concourse.mybir as mybir
    from concourse.bass import ts, ds
    from concourse.masks import make_identity

    dt = mybir.dt
    AF = mybir.ActivationFunctionType

    nc = bacc.Bacc("TRN2", target_bir_lowering=False, debug=False)

    xqT = nc.dram_tensor("xqT", [DM, S], dt.bfloat16, kind="ExternalInput").ap()
    xkT = nc.dram_tensor("xkT", [DM, S], dt.bfloat16, kind="ExternalInput").ap()
    xvT = nc.dram_tensor("xvT", [DM, S], dt.bfloat16, kind="ExternalInput").ap()
    wcat = nc.dram_tensor("wcat", [DM, WCOLS], dt.bfloat16,
                          kind="ExternalInput").ap()
    wo = nc.dram_tensor("wo", [HPC * HD, DM], dt.bfloat16,
                        kind="ExternalInput").ap()
    bcat = nc.dram_tensor("bcat", [WCOLS], dt.float32, kind="ExternalInput").ap()
    out_pT = nc.dram_tensor("out_pT", [DM, S], dt.bfloat16,
                            kind="ExternalOutput").ap()

    with tile.TileContext(nc) as tc:
        def body():
          with tc.tile_pool(name="consts", bufs=1) as cpool:
            wc_sb = cpool.tile([128, KT * WCOLS], dt.bfloat16, tag="wc")
            nc.sync.dma_start(
                out=wc_sb.rearrange("p (k c) -> p k c", k=KT),
                in_=wcat.rearrange("(k p) c -> p k c", p=128))
            wo_sb = cpool.tile([HD, HPC * DM], dt.bfloat16, tag="wos")
            nc.sync.dma_start(
                out=wo_sb.rearrange("d (h e) -> d h e", h=HPC),
                in_=wo.rearrange("(h d) e -> d h e", d=HD))
            bqk_sb = cpool.tile([128, WCOLS // 128], dt.float32, tag="bqk")
            nc.sync.dma_start(out=bqk_sb,
                              in_=bcat.rearrange("(m p) -> p m", p=128))
            ident = cpool.tile([128, 128], dt.bfloat16, tag="ident")
            make_identity(nc, ident)
            warm = cpool.tile([1, 16], dt.float32, tag="warm")
            nc.vector.memset(warm, 0.0)
            nc.scalar.activation(warm, warm, AF.Exp, scale=1.0)

            # inputs: one DMA per k-tile, issued after weights so the first
            # projection matmuls are never blocked on late weight loads
            # qk inputs in half-tiles so the first projections finish after
            # ~3MB of stream instead of 6MB; xv full tiles, streamed last
            xq_t, xk_t, xv_t = [], [], []
            for half in range(2):
                for k in range(KT):
                    for lst, dram, nm in ((xq_t, xqT, "xq"), (xk_t, xkT, "xk")):
                        tl = cpool.tile([128, S // 2], dt.bfloat16,
                                        tag=f"{nm}{k}_{half}",
                                        name=f"{nm}{k}_{half}")
                        nc.sync.dma_start(
                            out=tl, in_=dram[ts(k, 128), ts(half, S // 2)])
                        if half == 0:
                            lst.append([tl])
                        else:
                            lst[k].append(tl)
            for k in range(KT):
                tl = cpool.tile([128, S], dt.bfloat16, tag=f"xv{k}",
                                name=f"xv{k}")
                nc.sync.dma_start(out=tl, in_=xvT[ts(k, 128), :])
                xv_t.append(tl)

            def xq_k(k, c):
                return xq_t[k][c // 2][:, ts(c % 2, CH)]

            def xk_k(k, c):
                return xk_t[k][c // 2][:, ts(c % 2, CH)]

            def xv_k(k, c):
                return xv_t[k][:, ts(c, CH)]

            def wc_k(k, c0, w):
                return wc_sb[:, k * WCOLS + c0: k * WCOLS + c0 + w]

            # projection outputs (d' on partitions)
            qk01_sb = cpool.tile([128, S], dt.bfloat16, tag="qk01")
            k01_sb = cpool.tile([128, S], dt.bfloat16, tag="k01")
            qk2_sb = cpool.tile([128, S], dt.bfloat16, tag="qk2")
            kT2_sb = cpool.tile([HD, S], dt.bfloat16, tag="kT2")
            v01_sb = cpool.tile([128, S], dt.bfloat16, tag="v01")
            v2_sb = cpool.tile([HD, S], dt.bfloat16, tag="v2")
            # V~ tiles: vv01[t] [128, 130] = [V_h0|1|V_h1|1]; vv2[t] [128, 65]
            vv01 = [cpool.tile([128, 2 * (HD + 1)], dt.bfloat16,
                               tag=f"vv01_{t}", name=f"vv01_{t}")
                    for t in range(ST)]
            vv2 = [cpool.tile([128, HD + 1], dt.bfloat16,
                              tag=f"vv2_{t}", name=f"vv2_{t}")
                   for t in range(ST)]
            onrm_sb = [cpool.tile([HD, S], dt.bfloat16, tag=f"onrm{h}",
                                  name=f"onrm{h}")
                       for h in range(HPC)]

            # ---------------- projections ----------------
            # Phase P1: V (+ Q2/K2) projections and V transposes first, so
            # attention for h0 can start as soon as Q01/K01 (phase P2) land.
            def proj_group(pool, c, col0, m, xf, row0, pi, tag=None, bufs=1):
                pt = pool.tile([128, CH], dt.float32, tag=tag or f"pp{pi}",
                               name=f"pp{pi}", bufs=bufs)
                for k in range(KT):
                    nc.tensor.matmul(
                        pt[row0:row0 + m, :],
                        wc_k(k, col0, m),
                        xf(k, c),
                        start=(k == 0),
                        stop=(k == KT - 1),
                    )
                return pt

            def bias_copy(pt, dst, c, bi):
                rows = dst.shape[0]
                nc.vector.tensor_scalar_add(
                    dst[:, ts(c, CH)], pt[0:rows, :], bqk_sb[0:rows, bi:bi + 1])

            def emit_vpath_chunk(pq1, ptr, c):
                p2 = proj_group(pq1, c, 256, 64, xq_k, 0, 2, tag="ppv",
                                bufs=2)
                for k in range(KT):
                    nc.tensor.matmul(
                        p2[64:128, :],
                        wc_k(k, 320, 64),
                        xk_k(k, c),
                        start=(k == 0),
                        stop=(k == KT - 1),
                    )
                bias_copy(p2, qk2_sb, c, 2)
                p3 = proj_group(pq1, c, 384, 128, xv_k, 0, 3, tag="ppv",
                                bufs=2)
                bias_copy(p3, v01_sb, c, 3)
                p4 = proj_group(pq1, c, 512, 64, xv_k, 0, 4, tag="ppv",
                                bufs=2)
                bias_copy(p4, v2_sb, c, 4)
                for t in range(4 * c, 4 * c + 4):
                    tr1 = ptr.tile([128, 128], dt.bfloat16, tag="ppv",
                                   name="tr1", bufs=2)
                    nc.tensor.transpose(tr1, v01_sb[:, ts(t, 128)], ident)
                    nc.vector.tensor_copy(
                        vv01[t].rearrange("p (h x) -> p h x",
                                          h=2)[:, :, 0:HD],
                        tr1.rearrange("p (h x) -> p h x", h=2),
                    )
                    nc.gpsimd.memset(
                        vv01[t].rearrange("p (h x) -> p h x",
                                          h=2)[:, :, HD:HD + 1],
                        1.0)
                    tr2 = ptr.tile([128, HD], dt.bfloat16, tag="ppv",
                                   name="tr2", bufs=2)
                    nc.tensor.transpose(tr2, v2_sb[:, ts(t, 128)],
                                        ident[0:HD, 0:HD])
                    nc.vector.tensor_copy(vv2[t][:, 0:HD], tr2)
                    nc.gpsimd.memset(vv2[t][:, HD:HD + 1], 1.0)

            qk_heads = [
                (qk01_sb[0:HD, :], k01_sb[0:HD, :]),
                (qk01_sb[HD:128, :], k01_sb[HD:128, :]),
                (qk2_sb[0:HD, :], kT2_sb),
            ]
            vv_heads = [
                lambda t: vv01[t][:, 0:HD + 1],
                lambda t: vv01[t][:, HD + 1:2 * (HD + 1)],
                lambda t: vv2[t],
            ]

            # Phase A: QK01 projections interleaved with head-0 half-0
            # scores/exp (ACT starts while projections still stream)
            def sc_exp(h, t, half, psp, epool, ps_bufs=2, etag="exp",
                       ebufs=4):
                qT_h, kT_h = qk_heads[h]
                ps = psp.tile([128, 2 * CH], dt.float32, tag="ps", name="ps",
                              bufs=ps_bufs)
                for j in range(2):
                    c = 2 * half + j
                    nc.tensor.matmul(
                        ps[:, ts(j, CH)],
                        kT_h[:, ts(t, 128)],
                        qT_h[:, ts(c, CH)],
                    )
                et = epool.tile([128, 2 * CH], dt.bfloat16, tag=etag,
                                name="et", bufs=ebufs)
                nc.scalar.activation(et, ps, AF.Exp, scale=0.125)
                return et

            def pv(h, t, half, et, po, start=None, stop=None):
                for j in range(2):
                    c = 2 * half + j
                    nc.tensor.matmul(
                        po[c],
                        vv_heads[h](t),
                        et[:, ts(j, CH)],
                        start=(t == 0) if start is None else start,
                        stop=(t == ST - 1) if stop is None else stop,
                    )

            def norm_head(h, po, spool):
                for c in range(NCH):
                    dtile = spool.tile([1, CH], dt.float32, tag="den",
                                       name="dtile")
                    nc.vector.tensor_copy(dtile, po[c][HD:HD + 1, :])
                    rtile = spool.tile([1, CH], dt.float32, tag="rec",
                                       name="rtile")
                    nc.vector.reciprocal_approx_fast(out=rtile, in_=dtile)
                    bcst = spool.tile([HD, CH], dt.float32, tag="bcast",
                                      name="bcst")
                    nc.gpsimd.partition_broadcast(bcst, rtile)
                    dst = onrm_sb[h][:, ts(c, CH)]
                    nc.vector.tensor_mul(dst, po[c][0:HD, :], bcst)

            ets0 = {}
            ctx_epool = tc.tile_pool(name="expall", bufs=16)
            epool_all = ctx_epool.__enter__()
            with (
                tc.tile_pool(name="pproj2", bufs=1, space="PSUM") as pq2,
                tc.tile_pool(name="psA", bufs=1, space="PSUM") as psA,
            ):
                ready = {1: range(0, 4), 2: range(4, 8), 3: range(8, 16)}
                for c in range(NCH):
                    p0 = proj_group(pq2, c, 0, 128, xq_k, 0, 0)
                    bias_copy(p0, qk01_sb, c, 0)
                    p1 = proj_group(pq2, c, 128, 128, xk_k, 0, 1)
                    bias_copy(p1, k01_sb, c, 1)
                    for t in ready.get(c, ()):
                        ets0[t] = sc_exp(0, t, 0, psA, epool_all, ps_bufs=3,
                                         etag="et", ebufs=16)

            def norm_chunk(h, c, po_c, spool):
                dtile = spool.tile([1, CH], dt.float32, tag="den",
                                   name="dtile")
                nc.vector.tensor_copy(dtile, po_c[HD:HD + 1, :])
                rtile = spool.tile([1, CH], dt.float32, tag="rec",
                                   name="rtile")
                nc.vector.reciprocal_approx_fast(out=rtile, in_=dtile)
                bcst = spool.tile([HD, CH], dt.float32, tag="bcast",
                                  name="bcst")
                nc.gpsimd.partition_broadcast(bcst, rtile)
                nc.vector.tensor_mul(onrm_sb[h][:, ts(c, CH)],
                                     po_c[0:HD, :], bcst)

            def pv_half(h, t, half, et, poA, poB):
                for j, po_c in ((0, poA), (1, poB)):
                    nc.tensor.matmul(
                        po_c,
                        vv_heads[h](t),
                        et[:, ts(j, CH)],
                        start=(t == 0),
                        stop=(t == ST - 1),
                    )

            def outproj_chunk(c, pool, opool_t):
                for e in range(KT):
                    pout = pool.tile([128, CH], dt.float32, tag="pAB",
                                     name="pout", bufs=2)
                    for h in range(HPC):
                        nc.tensor.matmul(
                            pout,
                            wo_sb[:, h * DM + e * 128: h * DM + (e + 1) * 128],
                            onrm_sb[h][:, ts(c, CH)],
                            start=(h == 0),
                            stop=(h == HPC - 1),
                        )
                        
                    if c % 2 == 0:
                        nc.vector.tensor_copy(opool_t[e][:, ts(c, CH)], pout)
                    else:
                        nc.scalar.copy(opool_t[e][:, ts(c, CH)], pout)
                    if c == NCH - 1:
                        nc.sync.dma_start(out=out_pT[ts(e, 128), :],
                                          in_=opool_t[e])

            # output staging tiles (written chunk-wise, DMA'd when complete)
            ot_tiles = [cpool.tile([128, S], dt.bfloat16, tag=f"ot{e}",
                                   name=f"ot{e}") for e in range(KT)]

            # Phase B: V-path + head-0 half-0 PV + head-1 half-0
            # (fills the ACT-idle window while PE does the V-path)
            with (
                tc.tile_pool(name="pvb", bufs=1, space="PSUM") as pvb,
                tc.tile_pool(name="smallsB", bufs=4) as spoolB,
            ):
                po00 = pvb.tile([HD + 1, CH], dt.float32, tag="po0",
                                name="po00")
                po01 = pvb.tile([HD + 1, CH], dt.float32, tag="po1",
                                name="po01")
                po10 = pvb.tile([HD + 1, CH], dt.float32, tag="po2",
                                name="po10")
                po11 = pvb.tile([HD + 1, CH], dt.float32, tag="po3",
                                name="po11")
                with tc.tile_pool(name="ppv", bufs=1, space="PSUM") as ppvp:
                    eth1 = {}
                    for c in range(NCH):
                        emit_vpath_chunk(ppvp, ppvp, c)
                        for t in range(4 * c, 4 * c + 4):
                            pv_half(0, t, 0, ets0.pop(t), po00, po01)
                        for t in range(4 * c, 4 * c + 4):
                            e1 = sc_exp(1, t, 0, ppvp, epool_all,
                                        ps_bufs=1, etag="et", ebufs=16)
                            pv_half(1, t, 0, e1, po10, po11)
                    nc.sync.dma_start(out=kT2_sb, in_=qk2_sb[64:128, :])
                norm_chunk(0, 0, po00, spoolB)
                norm_chunk(0, 1, po01, spoolB)
                norm_chunk(1, 0, po10, spoolB)
                norm_chunk(1, 1, po11, spoolB)

            # Phases C/D: software-pipelined half-major attention
            with (
                tc.tile_pool(name="pCD", bufs=1, space="PSUM") as pcd,
                tc.tile_pool(name="smalls", bufs=4) as spool,
            ):
                epool = epool_all
                def half_loop(h, half):
                    poA = pcd.tile([HD + 1, CH], dt.float32, tag="pAB",
                                   name="poA", bufs=2)
                    poB = pcd.tile([HD + 1, CH], dt.float32, tag="pAB",
                                   name="poB", bufs=2)
                    ets = {}
                    LAG = 3
                    for t in range(ST):
                        ets[t] = sc_exp(h, t, half, pcd, epool, ps_bufs=3,
                                        etag="et", ebufs=16)
                        if t >= LAG:
                            pv_half(h, t - LAG, half, ets.pop(t - LAG),
                                    poA, poB)
                    for t in range(ST - LAG, ST):
                        pv_half(h, t, half, ets.pop(t), poA, poB)
                    norm_chunk(h, 2 * half, poA, spool)
                    norm_chunk(h, 2 * half + 1, poB, spool)

                half_loop(0, 1)
                half_loop(1, 1)
                half_loop(2, 0)
                # overlap first half of output projection with h2's second half
                outproj_chunk(0, pcd, ot_tiles)
                half_loop(2, 1)
                outproj_chunk(1, pcd, ot_tiles)
                outproj_chunk(2, pcd, ot_tiles)
                outproj_chunk(3, pcd, ot_tiles)
            ctx_epool.__exit__(None, None, None)

        if loop_reps > 1:
            with tc.For_i(0, loop_reps, 1):
                body()
        else:
            body()

    nc.compile()
    return nc


def _shard_inputs(query, key, value, wq, bq, wk, bk, wv, bv, wo, bo):
    """Build the 8 per-core input maps."""
    f32 = np.float32
    in_maps = []
    for core in range(NCORES):
        b = core // 4
        h0 = (core % 4) * HPC
        cs = slice(h0 * HD, (h0 + HPC) * HD)
        wq_s = np.asarray(wq[:, cs], f32)
        wk_s = np.asarray(wk[:, cs], f32)
        wv_s = np.asarray(wv[:, cs], f32)
        pad = np.zeros((DM, HD), f32)
        wcat = np.concatenate(
            [wq_s[:, 0:128], wk_s[:, 0:128], wq_s[:, 128:192],
             wk_s[:, 128:192], wv_s[:, 0:128], wv_s[:, 128:192], pad], axis=1)
        bq_s, bk_s, bv_s = (np.asarray(x[cs], f32) for x in (bq, bk, bv))
        bcat = np.concatenate([bq_s[0:128], bk_s[0:128], bq_s[128:192],
                               bk_s[128:192], bv_s[0:128], bv_s[128:192],
                               np.zeros(64, f32)])
        in_maps.append({
            "xqT": np.ascontiguousarray(np.asarray(query, f32)[b].T).astype(BF16),
            "xkT": np.ascontiguousarray(np.asarray(key, f32)[b].T).astype(BF16),
            "xvT": np.ascontiguousarray(np.asarray(value, f32)[b].T).astype(BF16),
            "wcat": np.ascontiguousarray(wcat).astype(BF16),
            "wo": np.ascontiguousarray(np.asarray(wo, f32)[cs, :]).astype(BF16),
            "bcat": np.ascontiguousarray(bcat),
        })
    return in_maps


def kernel(query, key, value, wq, bq, wk, bk, wv, bv, wo, bo):
    global _compiled
    from concourse.bass_utils import run_bass_kernel_spmd

    if _compiled is None:
        _compiled = _build()
    nc = _compiled

    in_maps = _shard_inputs(query, key, value, wq, bq, wk, bk, wv, bv, wo, bo)
    res = run_bass_kernel_spmd(nc, in_maps, list(range(NCORES)))

    out = np.zeros((B, S, DM), dtype=np.float32)
    for core in range(NCORES):
        b = core // 4
        out[b] += res.results[core]["out_pT"].astype(np.float32).T
    corr = (np.asarray(bv, np.float64) @ np.asarray(wo, np.float64)
            + np.asarray(bo, np.float64)).astype(np.float32)
    out += corr[None, None, :]
    return out
